# revision 4
# baseline (speedup 1.0000x reference)
"""Multi-head attention (B=4, S=2048, D=1024, H=16) on 8 Trainium2 NeuronCores.

Sharding: core c handles batch c//2 and head-group c%2 (8 heads = 512 dims of
the per-head concat). Each core computes its q/k/v projections (tensor
parallel over heads), attention for its 8 heads, and a partial output
projection over its 512 concat dims; the host sums the two partials per batch.

The softmax exp on the ACT engine (33.5M elements/core at ~1 elem/cycle/lane
@1.2GHz) is the hard floor (~295us busy), so the whole kernel is scheduled as
one ACT-saturated attention pipeline:
  - all matmul operands bf16 (full-rate streaming, FWL weight loads that
    overlap in the PE's reorder window); psum stays fp32.
  - k/v projections are emitted just-in-time inside the first attention
    rounds (scores for k-chunk kc only need kT up to kc; ctx only needs v up
    to kc), so the ACT engine starts ~10us in instead of after a serial
    projection phase.
  - q/out projections feed a global background queue drained in the PE slack
    of later rounds; no forced drain at round boundaries.
  - psum tags: ctx accumulators ("acc") and transient projection psums ("pj")
    get separate slot rings so a projection psum never has to wait for a live
    ctx accumulator's round-end evacuation.
  - scores are computed transposed S^T[k, q] so the mask is a per-partition
    ACT bias and exp(scale*s + bias) is one ACT op per k-chunk (no max
    subtraction: |scores*scale| <= ~5 by construction).
  - ctx^T = [V | 1]^T @ P^T accumulated over k-chunks; psum row 64 is the
    softmax denominator (flash-style deferred normalization).
  - normalization: reciprocal of the denominator row, partition-broadcast on
    the (idle) GPSIMD engine, then one fused DVE pass does psum-evacuate +
    normalize + bf16 downcast.

Host epilogue: out[b] = partial[2b] + partial[2b+1] + (Wo @ bv + bo); the
value bias commutes with softmax (rows sum to 1) so it is exact. The key and
query biases are applied on-device.
"""

import sys

sys.path.insert(0, "/opt/trn_rl_repo")

import numpy as np

import concourse.bacc as bacc
import concourse.mybir as mybir
import concourse.tile as tile
from concourse.bass_utils import run_bass_kernel_spmd

f32 = mybir.dt.float32
bf16 = mybir.dt.bfloat16
u16 = mybir.dt.uint16
AF = mybir.ActivationFunctionType
ALU = mybir.AluOpType

B, S, E, H = 4, 2048, 1024, 16
DH = E // H  # 64
G = E // 2  # 512 dims per core (8 heads)
HL = H // 2  # heads per core
EC = E // 128  # 8 e-chunks (projection contraction)
DC = G // 128  # 4 head-pairs per core
QT = S // 512  # 4 q-tiles
KC = S // 128  # 16 k-chunks
GC = G // 128  # 4 chunks of the local concat dim (out-proj contraction)
SCALE = 1.0 / np.sqrt(np.float64(E))
MASK_NEG = -88.0  # exp(-88 + |s|max) == 0 in fp32 for masked keys

_NC = None


def _build_program():
    nc = bacc.Bacc("TRN2", target_bir_lowering=False, debug=False, num_devices=8)

    xqT = nc.dram_tensor("xqT", [E, S], bf16, kind="ExternalInput").ap()
    xkT = nc.dram_tensor("xkT", [E, S], bf16, kind="ExternalInput").ap()
    xvT = nc.dram_tensor("xvT", [E, S], bf16, kind="ExternalInput").ap()
    wqT = nc.dram_tensor("wqT", [E, G], bf16, kind="ExternalInput").ap()
    wkT = nc.dram_tensor("wkT", [E, G], bf16, kind="ExternalInput").ap()
    wvT = nc.dram_tensor("wvT", [E, G], bf16, kind="ExternalInput").ap()
    woT = nc.dram_tensor("woT", [G, E], bf16, kind="ExternalInput").ap()
    bqd = nc.dram_tensor("bqd", [128, DC], f32, kind="ExternalInput").ap()
    bkd = nc.dram_tensor("bkd", [128, DC], f32, kind="ExternalInput").ap()
    maskb = nc.dram_tensor("maskb", [128, KC], f32, kind="ExternalInput").ap()
    out = nc.dram_tensor("out", [E, S], f32, kind="ExternalOutput").ap()  # transposed

    with tile.TileContext(nc) as tc:
        with (
            tc.tile_pool(name="weights", bufs=1) as wpool,
            tc.tile_pool(name="persist", bufs=1) as ppool,
            tc.tile_pool(name="xqstream", bufs=3) as xqstream,
            tc.tile_pool(name="xvstream", bufs=2) as xvstream,
            tc.tile_pool(name="qtile", bufs=2) as qpool,
            tc.tile_pool(name="exp", bufs=6) as epool,
            tc.tile_pool(name="norm", bufs=3) as npool,
            tc.tile_pool(name="outsb", bufs=4) as opool,
            tc.tile_pool(name="s_psum", bufs=2, space="PSUM") as s_psum,
            tc.tile_pool(name="c_psum", bufs=2, space="PSUM") as c_psum,
        ):
            # ---- persistent SBUF tensors ----
            kT_sb = ppool.tile([128, DC, S], bf16)
            v_sb = ppool.tile([128, KC, HL, DH + 1], bf16)
            ctxT_sb = ppool.tile([128, DC, S], bf16)
            xk_sb = ppool.tile([128, EC, S], bf16)  # resident key input
            wq_sb = wpool.tile([128, EC, G], bf16)
            wk_sb = wpool.tile([128, EC, G], bf16)
            wv_sb = wpool.tile([128, EC, G], bf16)
            wo_sb = wpool.tile([128, GC, E], bf16)
            bq_sb = wpool.tile([128, DC], f32)
            bk_sb = wpool.tile([128, DC], f32)
            mb_sb = wpool.tile([128, KC], f32)
            warm = wpool.tile([1, 8], f32)

            # ones column for the denominator fusion: preset whole tile, the
            # projection copies later overwrite cols 0..DH-1 of each head block
            nc.gpsimd.memset(v_sb[:], 1.0)
            nc.gpsimd.memset(warm[:], 0.0)
            # pre-load the exp table set during the DMA prologue (~2.7us)
            nc.scalar.activation(warm[:], warm[:], AF.Exp)

            # ---- input DMAs, most-urgent first ----
            nc.sync.dma_start(wk_sb[:], wkT.rearrange("(ec p) g -> p ec g", p=128))
            nc.sync.dma_start(bk_sb[:], bkd)
            nc.sync.dma_start(mb_sb[:], maskb)

            def xk_dma(st):
                nc.sync.dma_start(
                    xk_sb[:, :, st * 512 : (st + 1) * 512],
                    xkT[:, st * 512 : (st + 1) * 512].rearrange(
                        "(ec p) s -> p ec s", p=128
                    ),
                )

            xk_dma(0)
            nc.sync.dma_start(wq_sb[:], wqT.rearrange("(ec p) g -> p ec g", p=128))
            nc.sync.dma_start(bq_sb[:], bqd)
            nc.sync.dma_start(wv_sb[:], wvT.rearrange("(ec p) g -> p ec g", p=128))

            def xv_stream(sg):
                t = xvstream.tile([128, EC, 512], bf16, tag="xv", name=f"xv{sg}")
                nc.sync.dma_start(
                    t[:],
                    xvT[:, sg * 512 : (sg + 1) * 512].rearrange(
                        "(ec p) s -> p ec s", p=128
                    ),
                )
                return t

            # ---------------- work generators (yield ~per 2 matmuls) ----------------
            def kproj_chunk(st, dc):
                """kT for keys st*512..+512, head-pair dc: 8 matmuls + bias."""
                ps = c_psum.tile([128, 512], f32, tag="pj", name=f"kp{st}_{dc}")
                for ec in range(EC):
                    nc.tensor.matmul(
                        ps[:],
                        lhsT=wk_sb[:, ec, dc * 128 : (dc + 1) * 128],
                        rhs=xk_sb[:, ec, st * 512 : (st + 1) * 512],
                        start=(ec == 0),
                        stop=(ec == EC - 1),
                    )
                    if ec % 2 == 1:
                        yield
                nc.vector.tensor_add(
                    out=kT_sb[:, dc, st * 512 : (st + 1) * 512],
                    in0=ps[:],
                    in1=bk_sb[:, dc : dc + 1].to_broadcast((128, 512)),
                )

            def vproj_chunk(sc, xv_t):
                """v rows sc*128..+128 for all 8 heads: 8 matmuls + copy."""
                sci = sc % 4
                ps = c_psum.tile([128, 512], f32, tag="pj", name=f"vp{sc}")
                for ec in range(EC):
                    nc.tensor.matmul(
                        ps[:, :G],
                        lhsT=xv_t[:, ec, sci * 128 : (sci + 1) * 128],
                        rhs=wv_sb[:, ec, :],
                        start=(ec == 0),
                        stop=(ec == EC - 1),
                    )
                    if ec % 2 == 1:
                        yield
                nc.vector.tensor_copy(
                    out=v_sb[:, sc, :, 0:DH],
                    in_=ps[:, :G].rearrange("p (h d) -> p h d", h=HL),
                )

            qT_ts = {}
            xq_ts = {}

            def qproj_chunk(qt, dc):
                """one dc-chunk of the qT projection"""
                if dc == 0:
                    qT_ts[qt] = qpool.tile(
                        [128, DC, 512], bf16, tag="qT", name=f"qT{qt}"
                    )
                    t = xqstream.tile([128, EC, 512], bf16, tag="xq", name=f"xq{qt}")
                    nc.sync.dma_start(
                        t[:],
                        xqT[:, qt * 512 : (qt + 1) * 512].rearrange(
                            "(ec p) s -> p ec s", p=128
                        ),
                    )
                    xq_ts[qt] = t
                ps = c_psum.tile([128, 512], f32, tag="pj", name=f"qp{qt}_{dc}")
                for ec in range(EC):
                    nc.tensor.matmul(
                        ps[:],
                        lhsT=wq_sb[:, ec, dc * 128 : (dc + 1) * 128],
                        rhs=xq_ts[qt][:, ec, :],
                        start=(ec == 0),
                        stop=(ec == EC - 1),
                    )
                    if ec % 2 == 1:
                        yield
                nc.vector.tensor_add(
                    out=qT_ts[qt][:, dc, :],
                    in0=ps[:],
                    in1=bq_sb[:, dc : dc + 1].to_broadcast((128, 512)),
                )

            def outproj_chunk(st, ec):
                """one ec-chunk of the transposed output projection"""
                ps = c_psum.tile([128, 512], f32, tag="pj", name=f"op{st}_{ec}")
                for gc in range(GC):
                    nc.tensor.matmul(
                        ps[:],
                        lhsT=wo_sb[:, gc, ec * 128 : (ec + 1) * 128],
                        rhs=ctxT_sb[:, gc, st * 512 : (st + 1) * 512],
                        start=(gc == 0),
                        stop=(gc == GC - 1),
                    )
                    if gc % 2 == 1:
                        yield
                o_sb = opool.tile([128, 512], f32, tag="osb")
                nc.vector.tensor_copy(out=o_sb[:], in_=ps[:])
                nc.sync.dma_start(
                    out[ec * 128 : (ec + 1) * 128, st * 512 : (st + 1) * 512],
                    o_sb[:],
                )

            # global background queue: (key, generator); drained in PE slack
            bg = []

            def drive(n=1):
                while n > 0 and bg:
                    try:
                        next(bg[0][1])
                        n -= 1
                    except StopIteration:
                        bg.pop(0)

            def force(key):
                """run queued generators up to and including `key` to completion"""
                while any(k == key for k, _ in bg):
                    try:
                        next(bg[0][1])
                    except StopIteration:
                        bg.pop(0)

            # ---------------- prologue ----------------
            for _ in kproj_chunk(0, 0):
                pass
            for _ in qproj_chunk(0, 0):
                pass
            xv_ts = {0: xv_stream(0), 1: xv_stream(1)}
            xk_dma(1)
            for sc in range(4):
                for _ in vproj_chunk(sc, xv_ts[0]):
                    pass
            xk_dma(2)
            xk_dma(3)
            nc.sync.dma_start(wo_sb[:], woT.rearrange("(gc p) e -> p gc e", p=128))

            # ---------------- attention rounds ----------------
            for qt in range(QT):
                q0 = qt * 512
                for hp in range(DC):
                    rnd = qt * DC + hp
                    force(("qp", qt, hp))
                    if 1 <= rnd <= 3:
                        # JIT kproj for head-pair dc=hp, first key tile
                        for _ in kproj_chunk(0, hp):
                            pass
                    qT_t = qT_ts[qt]
                    ctx0 = c_psum.tile([128, 512], f32, tag="acc", name=f"c0_{rnd}")
                    ctx1 = c_psum.tile([128, 512], f32, tag="acc", name=f"c1_{rnd}")
                    # software-pipelined: ctx(kc-1) and background work are
                    # emitted BEFORE the scores pair of kc
                    pend = [None]

                    def ctx_pair(kc, ctx0=ctx0, ctx1=ctx1, hp=hp, pend=pend):
                        e = pend[0]
                        nc.tensor.matmul(
                            ctx0[0 : DH + 1, :],
                            lhsT=v_sb[:, kc, 2 * hp, :],
                            rhs=e[:, 0:512],
                            start=(kc == 0),
                            stop=(kc == KC - 1),
                        )
                        nc.tensor.matmul(
                            ctx1[0 : DH + 1, :],
                            lhsT=v_sb[:, kc, 2 * hp + 1, :],
                            rhs=e[:, 512:1024],
                            start=(kc == 0),
                            stop=(kc == KC - 1),
                        )

                    for kc in range(KC):
                        k0 = kc * 128
                        # ---- just-in-time production / background work ----
                        if rnd == 0:
                            if kc < 12:
                                st = 1 + kc // 4
                                if kc % 4 == 0:
                                    if st + 1 <= 3:
                                        xv_ts[st + 1] = xv_stream(st + 1)
                                    for _ in kproj_chunk(st, 0):
                                        pass
                                for _ in vproj_chunk(kc + 4, xv_ts[st]):
                                    pass
                            elif kc == 12:
                                bg.append((("qp", qt, 1), qproj_chunk(qt, 1)))
                                bg.append((("qp", qt, 2), qproj_chunk(qt, 2)))
                                bg.append((("qp", qt, 3), qproj_chunk(qt, 3)))
                            if kc >= 12:
                                drive(2)
                        elif 1 <= rnd <= 3:
                            # JIT kproj(dc=hp, st=1..3): spread over the 4 kc
                            # before the scores that need each key tile
                            if kc < 12:
                                st = 1 + kc // 4
                                if kc % 4 == 0:
                                    bg.insert(0, (("kp", hp, st), kproj_chunk(st, hp)))
                                drive(1)
                                if kc % 4 == 3:
                                    force(("kp", hp, st))
                            else:
                                drive(1)
                        else:
                            drive(1)
                        if kc > 0:
                            ctx_pair(kc - 1)
                        sp = s_psum.tile([128, 1024], f32, tag="sp")
                        nc.tensor.matmul(
                            sp[:, 0:512],
                            lhsT=kT_sb[0:64, hp, k0 : k0 + 128],
                            rhs=qT_t[0:64, hp, :],
                            start=True,
                            stop=True,
                        )
                        nc.tensor.matmul(
                            sp[:, 512:1024],
                            lhsT=kT_sb[64:128, hp, k0 : k0 + 128],
                            rhs=qT_t[64:128, hp, :],
                            start=True,
                            stop=True,
                        )
                        e = epool.tile([128, 1024], bf16, tag="exp")
                        nc.scalar.activation(
                            e[:], sp[:], AF.Exp,
                            bias=mb_sb[:, kc : kc + 1], scale=float(SCALE),
                        )
                        pend[0] = e
                    ctx_pair(KC - 1)

                    # queue follow-on work for later rounds' PE slack
                    if qt < QT - 1:
                        bg.append((("qp", qt + 1, hp), qproj_chunk(qt + 1, hp)))
                    if qt > 0:
                        bg.append(
                            (("op", qt - 1, 2 * hp), outproj_chunk(qt - 1, 2 * hp))
                        )
                        bg.append(
                            (
                                ("op", qt - 1, 2 * hp + 1),
                                outproj_chunk(qt - 1, 2 * hp + 1),
                            )
                        )

                    # ---- finalize: reciprocal-normalize into ctxT_sb ----
                    for hq, cpsum in ((0, ctx0), (1, ctx1)):
                        pb = 64 * hq
                        qs = slice(q0, q0 + 512)
                        den = npool.tile([1, 512], f32, tag="den")
                        nc.vector.tensor_copy(out=den[:], in_=cpsum[DH : DH + 1, :])
                        rec = npool.tile([1, 512], f32, tag="rec")
                        nc.vector.reciprocal_approx_fast(rec[:], den[:])
                        rb = npool.tile([128, 512], f32, tag="rb")
                        nc.gpsimd.partition_broadcast(rb[:], rec[:])
                        # fused: evacuate psum + normalize + downcast to bf16
                        nc.vector.scalar_tensor_tensor(
                            out=ctxT_sb[pb : pb + 64, hp, qs],
                            in0=cpsum[0:DH, :],
                            scalar=0.0,
                            in1=rb[pb : pb + 64, :],
                            op0=ALU.add,
                            op1=ALU.mult,
                        )

            # tail: remaining background work + output projection for qt=3
            while bg:
                drive(1)
            for ec in range(EC):
                for _ in outproj_chunk(QT - 1, ec):
                    pass

    nc.compile()
    return nc


def _prep_core_inputs(query, key, value, mask, Wq, bq, Wk, bk, Wv, Wo):
    """Per-core input maps: core c -> batch c//2, head-group c%2."""
    import ml_dtypes

    f = ml_dtypes.bfloat16
    maps = []
    for c in range(8):
        b, g = c // 2, c % 2
        lo = g * G
        mrow = mask[b, 0].astype(np.float64)
        maskb = np.where(mrow == 0, MASK_NEG, 0.0).reshape(KC, 128).T
        maps.append(
            {
                "xqT": np.ascontiguousarray(query[b].T).astype(f, copy=False),
                "xkT": np.ascontiguousarray(key[b].T).astype(f, copy=False),
                "xvT": np.ascontiguousarray(value[b].T).astype(f, copy=False),
                "wqT": np.ascontiguousarray(Wq[lo : lo + G].T).astype(f, copy=False),
                "wkT": np.ascontiguousarray(Wk[lo : lo + G].T).astype(f, copy=False),
                "wvT": np.ascontiguousarray(Wv[lo : lo + G].T).astype(f, copy=False),
                "woT": np.ascontiguousarray(Wo[:, lo : lo + G].T).astype(f, copy=False),
                "bqd": np.ascontiguousarray(bq[lo : lo + G].reshape(DC, 128).T).astype(np.float32),
                "bkd": np.ascontiguousarray(bk[lo : lo + G].reshape(DC, 128).T).astype(np.float32),
                "maskb": np.ascontiguousarray(maskb).astype(np.float32),
            }
        )
    return maps


def kernel(query, key, value, mask, Wq, bq, Wk, bk, Wv, bv, Wo, bo, _results=None):
    global _NC
    query = np.asarray(query, dtype=np.float32)
    key = np.asarray(key, dtype=np.float32)
    value = np.asarray(value, dtype=np.float32)
    mask = np.asarray(mask)
    Wq, bq = np.asarray(Wq, np.float32), np.asarray(bq, np.float32)
    Wk, bk = np.asarray(Wk, np.float32), np.asarray(bk, np.float32)
    Wv, bv = np.asarray(Wv, np.float32), np.asarray(bv, np.float32)
    Wo, bo = np.asarray(Wo, np.float32), np.asarray(bo, np.float32)

    if _NC is None:
        _NC = _build_program()
    in_maps = _prep_core_inputs(query, key, value, mask, Wq, bq, Wk, bk, Wv, Wo)
    res = run_bass_kernel_spmd(_NC, in_maps, core_ids=list(range(8)))
    if _results is not None:
        _results.append(res)

    # host epilogue: sum the two head-group partials; bv commutes with softmax
    # (rows sum to 1) so its contribution is Wo @ bv, plus the output bias bo.
    extra = (Wo.astype(np.float64) @ bv.astype(np.float64) + bo.astype(np.float64)).astype(
        np.float32
    )
    out = np.empty((B, S, E), dtype=np.float32)
    for b in range(B):
        out[b] = (
            res.results[2 * b]["out"] + res.results[2 * b + 1]["out"]
        ).T + extra
    return out


# revision 14
# speedup vs baseline: 1.1434x; 1.1434x over previous
"""Multi-head attention (B=4, S=2048, D=1024, H=16) on 8 Trainium2 NeuronCores.

Sharding: core c handles batch c//2 and head-group c%2 (8 heads = 512 dims of
the per-head concat). Each core computes its q/k/v projections (tensor
parallel over heads), attention for its 8 heads, and a partial output
projection over its 512 concat dims; the host sums the two partials per batch.

The softmax exp on the ACT engine (33.5M elements/core at ~1 elem/cycle/lane
@1.2GHz) is the hard floor (~295us busy), so the whole kernel is scheduled as
one ACT-saturated attention pipeline:
  - all matmul operands bf16 (full-rate streaming, FWL weight loads that
    overlap in the PE's reorder window); psum stays fp32.
  - k/v projections are emitted just-in-time inside the first attention
    rounds (scores for k-chunk kc only need kT up to kc; ctx only needs v up
    to kc), so the ACT engine starts ~10us in instead of after a serial
    projection phase.
  - q/out projections feed a global background queue drained in the PE slack
    of later rounds; no forced drain at round boundaries.
  - psum tags: ctx accumulators ("acc") and transient projection psums ("pj")
    get separate slot rings so a projection psum never has to wait for a live
    ctx accumulator's round-end evacuation.
  - scores are computed transposed S^T[k, q] so the mask is a per-partition
    ACT bias and exp(scale*s + bias) is one ACT op per k-chunk (no max
    subtraction: |scores*scale| <= ~5 by construction).
  - ctx^T = [V | 1]^T @ P^T accumulated over k-chunks; psum row 64 is the
    softmax denominator (flash-style deferred normalization).
  - normalization: reciprocal of the denominator row, partition-broadcast on
    the (idle) GPSIMD engine, then one fused DVE pass does psum-evacuate +
    normalize + bf16 downcast.

Host epilogue: out[b] = partial[2b] + partial[2b+1] + (Wo @ bv + bo); the
value bias commutes with softmax (rows sum to 1) so it is exact. The key and
query biases are applied on-device.
"""

import sys

sys.path.insert(0, "/opt/trn_rl_repo")

import numpy as np

import concourse.bacc as bacc
import concourse.mybir as mybir
import concourse.tile as tile
from concourse.bass_utils import run_bass_kernel_spmd

f32 = mybir.dt.float32
f32r = mybir.dt.float32r
bf16 = mybir.dt.bfloat16
u16 = mybir.dt.uint16
AF = mybir.ActivationFunctionType
ALU = mybir.AluOpType

B, S, E, H = 4, 2048, 1024, 16
DH = E // H  # 64
G = E // 2  # 512 dims per core (8 heads)
HL = H // 2  # heads per core
EC = E // 128  # 8 e-chunks (projection contraction)
DC = G // 128  # 4 head-pairs per core
QT = S // 512  # 4 q-tiles
KC = S // 128  # 16 k-chunks
GC = G // 128  # 4 chunks of the local concat dim (out-proj contraction)
SCALE = 1.0 / np.sqrt(np.float64(E))
MASK_NEG = -88.0  # exp(-88 + |s|max) == 0 in fp32 for masked keys

_NC = None


def _build_program():
    nc = bacc.Bacc("TRN2", target_bir_lowering=False, debug=False, num_devices=8)

    xqT = nc.dram_tensor("xqT", [E, S], bf16, kind="ExternalInput").ap()
    xkT = nc.dram_tensor("xkT", [E, S], bf16, kind="ExternalInput").ap()
    xvT = nc.dram_tensor("xvT", [E, S], bf16, kind="ExternalInput").ap()
    wqT = nc.dram_tensor("wqT", [E, G], bf16, kind="ExternalInput").ap()
    wkT = nc.dram_tensor("wkT", [E, G], bf16, kind="ExternalInput").ap()
    wvT = nc.dram_tensor("wvT", [E, G], bf16, kind="ExternalInput").ap()
    woT = nc.dram_tensor("woT", [G, E], bf16, kind="ExternalInput").ap()
    bqd = nc.dram_tensor("bqd", [128, DC], f32, kind="ExternalInput").ap()
    bkd = nc.dram_tensor("bkd", [128, DC], f32, kind="ExternalInput").ap()
    maskb = nc.dram_tensor("maskb", [128, KC], f32, kind="ExternalInput").ap()
    out = nc.dram_tensor("out", [E, S], f32, kind="ExternalOutput").ap()  # transposed

    with tile.TileContext(nc) as tc:
        with (
            tc.tile_pool(name="weights", bufs=1) as wpool,
            tc.tile_pool(name="persist", bufs=1) as ppool,
            tc.tile_pool(name="xqstream", bufs=2) as xqstream,
            tc.tile_pool(name="xvstream", bufs=2) as xvstream,
            tc.tile_pool(name="qtile", bufs=2) as qpool,
            tc.tile_pool(name="exp", bufs=3) as epool,
            tc.tile_pool(name="norm", bufs=3) as npool,
            tc.tile_pool(name="outsb", bufs=2) as opool,
            tc.tile_pool(name="s_psum", bufs=2, space="PSUM") as s_psum,
            tc.tile_pool(name="c_psum", bufs=2, space="PSUM") as c_psum,
        ):
            # ---- persistent SBUF tensors ----
            kT_sb = ppool.tile([128, DC, S], bf16)
            v_sb = ppool.tile([128, KC, HL, DH + 1], f32r)
            ctxT_sb = ppool.tile([128, DC, S], bf16)
            xk_sb = ppool.tile([128, EC, S], bf16)  # resident key input
            o_acc = ppool.tile([128, EC, 512], bf16)  # qt3 out-proj partials
            wq_sb = wpool.tile([128, EC, G], bf16)
            wk_sb = wpool.tile([128, EC, G], bf16)
            wv_sb = wpool.tile([128, EC, G], bf16)
            wo_sb = wpool.tile([128, GC, E], bf16)
            bq_sb = wpool.tile([128, DC], f32)
            bk_sb = wpool.tile([128, DC], f32)
            mb_sb = wpool.tile([128, KC], f32)
            warm = wpool.tile([1, 8], f32)

            # ones column for the denominator fusion: preset whole tile, the
            # projection copies later overwrite cols 0..DH-1 of each head block
            nc.gpsimd.memset(v_sb[:].bitcast(f32), 1.0)
            nc.gpsimd.memset(warm[:], 0.0)
            # pre-load the exp table set during the DMA prologue (~2.7us)
            nc.scalar.activation(warm[:], warm[:], AF.Exp)

            # ---- input DMAs, most-urgent first ----
            nc.sync.dma_start(wk_sb[:], wkT.rearrange("(ec p) g -> p ec g", p=128))
            nc.sync.dma_start(bk_sb[:], bkd)
            nc.sync.dma_start(mb_sb[:], maskb)

            def xk_dma(st):
                nc.sync.dma_start(
                    xk_sb[:, :, st * 512 : (st + 1) * 512],
                    xkT[:, st * 512 : (st + 1) * 512].rearrange(
                        "(ec p) s -> p ec s", p=128
                    ),
                )

            xk_dma(0)
            nc.sync.dma_start(wq_sb[:], wqT.rearrange("(ec p) g -> p ec g", p=128))
            nc.sync.dma_start(bq_sb[:], bqd)
            nc.sync.dma_start(wv_sb[:], wvT.rearrange("(ec p) g -> p ec g", p=128))

            def xv_stream(sg):
                t = xvstream.tile([128, EC, 512], bf16, tag="xv", name=f"xv{sg}")
                nc.sync.dma_start(
                    t[:],
                    xvT[:, sg * 512 : (sg + 1) * 512].rearrange(
                        "(ec p) s -> p ec s", p=128
                    ),
                )
                return t

            # ---------------- work generators (yield ~per 2 matmuls) ----------------
            def kproj_chunk(st, dc):
                """kT for keys st*512..+512, head-pair dc: 8 matmuls + bias."""
                ps = c_psum.tile([128, 512], f32, tag="pj", name=f"kp{st}_{dc}")
                for ec in range(EC):
                    nc.tensor.matmul(
                        ps[:],
                        lhsT=wk_sb[:, ec, dc * 128 : (dc + 1) * 128],
                        rhs=xk_sb[:, ec, st * 512 : (st + 1) * 512],
                        start=(ec == 0),
                        stop=(ec == EC - 1),
                    )
                    if ec % 2 == 1:
                        yield
                nc.vector.tensor_add(
                    out=kT_sb[:, dc, st * 512 : (st + 1) * 512],
                    in0=ps[:],
                    in1=bk_sb[:, dc : dc + 1].to_broadcast((128, 512)),
                )

            def vproj_chunk(sc, xv_t):
                """v rows sc*128..+128 for all 8 heads: 8 matmuls + copy."""
                sci = sc % 4
                ps = c_psum.tile([128, 512], f32, tag="pj", name=f"vp{sc}")
                for ec in range(EC):
                    nc.tensor.matmul(
                        ps[:, :G],
                        lhsT=xv_t[:, ec, sci * 128 : (sci + 1) * 128],
                        rhs=wv_sb[:, ec, :],
                        start=(ec == 0),
                        stop=(ec == EC - 1),
                    )
                    if ec % 2 == 1:
                        yield
                nc.vector.tensor_copy(
                    out=v_sb[:, sc, :, 0:DH],
                    in_=ps[:, :G].rearrange("p (h d) -> p h d", h=HL),
                )

            qT_ts = {}
            xq_ts = {}

            def qproj_chunk(qt, dc):
                """one dc-chunk of the qT projection"""
                if dc == 0:
                    qT_ts[qt] = qpool.tile(
                        [128, DC, 512], bf16, tag="qT", name=f"qT{qt}"
                    )
                    t = xqstream.tile([128, EC, 512], bf16, tag="xq", name=f"xq{qt}")
                    nc.sync.dma_start(
                        t[:],
                        xqT[:, qt * 512 : (qt + 1) * 512].rearrange(
                            "(ec p) s -> p ec s", p=128
                        ),
                    )
                    xq_ts[qt] = t
                ps = c_psum.tile([128, 512], f32, tag="pj", name=f"qp{qt}_{dc}")
                for ec in range(EC):
                    nc.tensor.matmul(
                        ps[:],
                        lhsT=wq_sb[:, ec, dc * 128 : (dc + 1) * 128],
                        rhs=xq_ts[qt][:, ec, :],
                        start=(ec == 0),
                        stop=(ec == EC - 1),
                    )
                    if ec % 2 == 1:
                        yield
                nc.vector.tensor_add(
                    out=qT_ts[qt][:, dc, :],
                    in0=ps[:],
                    in1=bq_sb[:, dc : dc + 1].to_broadcast((128, 512)),
                )

            def outproj_chunk(st, ec):
                """one ec-chunk of the transposed output projection"""
                ps = c_psum.tile([128, 512], f32, tag="pj", name=f"op{st}_{ec}")
                for gc in range(GC):
                    nc.tensor.matmul(
                        ps[:],
                        lhsT=wo_sb[:, gc, ec * 128 : (ec + 1) * 128],
                        rhs=ctxT_sb[:, gc, st * 512 : (st + 1) * 512],
                        start=(gc == 0),
                        stop=(gc == GC - 1),
                    )
                    if gc % 2 == 1:
                        yield
                o_sb = opool.tile([128, 512], f32, tag="osb")
                nc.vector.tensor_copy(out=o_sb[:], in_=ps[:])
                nc.sync.dma_start(
                    out[ec * 128 : (ec + 1) * 128, st * 512 : (st + 1) * 512],
                    o_sb[:],
                )

            def o3_partial(hp):
                """qt3 out-projection: one head-pair's contribution, SBUF-accumulated"""
                q3 = slice((QT - 1) * 512, QT * 512)
                for ec in range(EC):
                    ps = c_psum.tile([128, 512], f32, tag="pj", name=f"o3p{hp}_{ec}")
                    nc.tensor.matmul(
                        ps[:],
                        lhsT=wo_sb[:, hp, ec * 128 : (ec + 1) * 128],
                        rhs=ctxT_sb[:, hp, q3],
                        start=True,
                        stop=True,
                    )
                    yield
                    if hp == 0:
                        nc.vector.tensor_copy(out=o_acc[:, ec, :], in_=ps[:])
                    elif hp < DC - 1:
                        nc.vector.tensor_add(
                            out=o_acc[:, ec, :], in0=ps[:], in1=o_acc[:, ec, :]
                        )
                    else:
                        o_sb = opool.tile([128, 512], f32, tag="osb")
                        nc.vector.scalar_tensor_tensor(
                            out=o_sb[:],
                            in0=ps[:],
                            scalar=0.0,
                            in1=o_acc[:, ec, :],
                            op0=ALU.add,
                            op1=ALU.add,
                        )
                        nc.sync.dma_start(
                            out[ec * 128 : (ec + 1) * 128, q3], o_sb[:]
                        )
                    yield

            # global background queue: (key, generator); drained in PE slack
            bg = []

            def drive(n=1):
                while n > 0 and bg:
                    try:
                        next(bg[0][1])
                        n -= 1
                    except StopIteration:
                        bg.pop(0)

            def force(key):
                """run queued generators up to and including `key` to completion"""
                while any(k == key for k, _ in bg):
                    try:
                        next(bg[0][1])
                    except StopIteration:
                        bg.pop(0)

            # ---------------- prologue ----------------
            for _ in kproj_chunk(0, 0):
                pass
            for _ in qproj_chunk(0, 0):
                pass
            xv_ts = {0: xv_stream(0), 1: xv_stream(1)}
            xk_dma(1)
            for sc in range(4):
                for _ in vproj_chunk(sc, xv_ts[0]):
                    pass
            xk_dma(2)
            xk_dma(3)
            nc.sync.dma_start(wo_sb[:], woT.rearrange("(gc p) e -> p gc e", p=128))

            # ---------------- attention rounds ----------------
            for qt in range(QT):
                q0 = qt * 512
                for hp in range(DC):
                    rnd = qt * DC + hp
                    force(("qp", qt, hp))
                    if 1 <= rnd <= 3:
                        # JIT kproj for head-pair dc=hp, first key tile
                        for _ in kproj_chunk(0, hp):
                            pass
                    qT_t = qT_ts[qt]
                    ctx0 = c_psum.tile([128, 512], f32, tag="acc", name=f"c0_{rnd}")
                    ctx1 = c_psum.tile([128, 512], f32, tag="acc", name=f"c1_{rnd}")
                    # software-pipelined: ctx(kc-1) and background work are
                    # emitted BEFORE the scores pair of kc
                    pend = [None]

                    def ctx_pair(kc, ctx0=ctx0, ctx1=ctx1, hp=hp, pend=pend):
                        e = pend[0]
                        nc.tensor.matmul(
                            ctx0[0 : DH + 1, :],
                            lhsT=v_sb[:, kc, 2 * hp, :],
                            rhs=e[:, 0:512],
                            start=(kc == 0),
                            stop=(kc == KC - 1),
                        )
                        nc.tensor.matmul(
                            ctx1[0 : DH + 1, :],
                            lhsT=v_sb[:, kc, 2 * hp + 1, :],
                            rhs=e[:, 512:1024],
                            start=(kc == 0),
                            stop=(kc == KC - 1),
                        )

                    for kc in range(KC):
                        k0 = kc * 128
                        # ---- just-in-time production / background work ----
                        if rnd == 0:
                            if kc < 12:
                                st = 1 + kc // 4
                                if kc % 4 == 0:
                                    if st + 1 <= 3:
                                        xv_ts[st + 1] = xv_stream(st + 1)
                                    for _ in kproj_chunk(st, 0):
                                        pass
                                for _ in vproj_chunk(kc + 4, xv_ts[st]):
                                    pass
                            elif kc == 12:
                                bg.append((("qp", qt, 1), qproj_chunk(qt, 1)))
                                bg.append((("qp", qt, 2), qproj_chunk(qt, 2)))
                                bg.append((("qp", qt, 3), qproj_chunk(qt, 3)))
                            if kc >= 12:
                                drive(2)
                        elif 1 <= rnd <= 3:
                            # JIT kproj(dc=hp, st=1..3): spread over the 4 kc
                            # before the scores that need each key tile
                            if kc < 12:
                                st = 1 + kc // 4
                                if kc % 4 == 0:
                                    bg.insert(0, (("kp", hp, st), kproj_chunk(st, hp)))
                                drive(1)
                                if kc % 4 == 3:
                                    force(("kp", hp, st))
                            else:
                                drive(1)
                        else:
                            drive(1)
                        if kc > 0:
                            ctx_pair(kc - 1)
                        sp = s_psum.tile([128, 1024], f32, tag="sp")
                        nc.tensor.matmul(
                            sp[:, 0:512],
                            lhsT=kT_sb[0:64, hp, k0 : k0 + 128],
                            rhs=qT_t[0:64, hp, :],
                            start=True,
                            stop=True,
                        )
                        nc.tensor.matmul(
                            sp[:, 512:1024],
                            lhsT=kT_sb[64:128, hp, k0 : k0 + 128],
                            rhs=qT_t[64:128, hp, :],
                            start=True,
                            stop=True,
                        )
                        e = epool.tile([128, 1024], f32r, tag="exp")
                        nc.scalar.activation(
                            e[:], sp[:], AF.Exp,
                            bias=mb_sb[:, kc : kc + 1], scale=float(SCALE),
                        )
                        pend[0] = e
                    ctx_pair(KC - 1)

                    # queue follow-on work for later rounds' PE slack
                    if qt < QT - 1:
                        bg.append((("qp", qt + 1, hp), qproj_chunk(qt + 1, hp)))
                    if qt > 0:
                        bg.append(
                            (("op", qt - 1, 2 * hp), outproj_chunk(qt - 1, 2 * hp))
                        )
                        bg.append(
                            (
                                ("op", qt - 1, 2 * hp + 1),
                                outproj_chunk(qt - 1, 2 * hp + 1),
                            )
                        )

                    # ---- finalize ----
                    # evacuate psum immediately (frees the acc slot for the
                    # next round), then reciprocal-normalize off-critical-path
                    for hq, cpsum in ((0, ctx0), (1, ctx1)):
                        pb = 64 * hq
                        qs = slice(q0, q0 + 512)
                        # pure-SBUF DVE ops must be lane-aligned: stage head
                        # hq's ctx on partitions pb..pb+64 (PSUM sources may
                        # cross partitions; SBUF ones may not)
                        cx = npool.tile(
                            [128, 512], f32, tag="cx", bufs=3, name=f"cx{rnd}_{hq}"
                        )
                        nc.vector.tensor_copy(
                            out=cx[pb : pb + DH, :], in_=cpsum[0:DH, :]
                        )
                        den = npool.tile([1, 512], f32, tag="dr", bufs=3, name="den")
                        nc.vector.tensor_copy(out=den[:], in_=cpsum[DH : DH + 1, :])
                        rec = npool.tile([1, 512], f32, tag="dr", bufs=3, name="rec")
                        nc.vector.reciprocal_approx_fast(rec[:], den[:])
                        rb = npool.tile([128, 512], f32, tag="rb", bufs=1)
                        nc.gpsimd.partition_broadcast(rb[:], rec[:])
                        # fused: normalize + downcast to bf16
                        nc.vector.scalar_tensor_tensor(
                            out=ctxT_sb[pb : pb + 64, hp, qs],
                            in0=cx[pb : pb + DH, :],
                            scalar=0.0,
                            in1=rb[pb : pb + 64, :],
                            op0=ALU.add,
                            op1=ALU.mult,
                        )
                    # qt3: out-projection partial per head-pair so the tail
                    # doesn't serialize 32 matmuls after the last round
                    if qt == QT - 1 and hp < DC - 1:
                        bg.append((("o3", hp), o3_partial(hp)))

            # tail: remaining background work + last head-pair of qt3 out-proj
            while bg:
                drive(1)
            for _ in o3_partial(DC - 1):
                pass

    nc.compile()
    return nc


def _prep_core_inputs(query, key, value, mask, Wq, bq, Wk, bk, Wv, Wo):
    """Per-core input maps: core c -> batch c//2, head-group c%2."""
    import ml_dtypes

    f = ml_dtypes.bfloat16
    maps = []
    for c in range(8):
        b, g = c // 2, c % 2
        lo = g * G
        mrow = mask[b, 0].astype(np.float64)
        maskb = np.where(mrow == 0, MASK_NEG, 0.0).reshape(KC, 128).T
        maps.append(
            {
                "xqT": np.ascontiguousarray(query[b].T).astype(f, copy=False),
                "xkT": np.ascontiguousarray(key[b].T).astype(f, copy=False),
                "xvT": np.ascontiguousarray(value[b].T).astype(f, copy=False),
                "wqT": np.ascontiguousarray(Wq[lo : lo + G].T).astype(f, copy=False),
                "wkT": np.ascontiguousarray(Wk[lo : lo + G].T).astype(f, copy=False),
                "wvT": np.ascontiguousarray(Wv[lo : lo + G].T).astype(f, copy=False),
                "woT": np.ascontiguousarray(Wo[:, lo : lo + G].T).astype(f, copy=False),
                "bqd": np.ascontiguousarray(bq[lo : lo + G].reshape(DC, 128).T).astype(np.float32),
                "bkd": np.ascontiguousarray(bk[lo : lo + G].reshape(DC, 128).T).astype(np.float32),
                "maskb": np.ascontiguousarray(maskb).astype(np.float32),
            }
        )
    return maps


def kernel(query, key, value, mask, Wq, bq, Wk, bk, Wv, bv, Wo, bo, _results=None):
    global _NC
    query = np.asarray(query, dtype=np.float32)
    key = np.asarray(key, dtype=np.float32)
    value = np.asarray(value, dtype=np.float32)
    mask = np.asarray(mask)
    Wq, bq = np.asarray(Wq, np.float32), np.asarray(bq, np.float32)
    Wk, bk = np.asarray(Wk, np.float32), np.asarray(bk, np.float32)
    Wv, bv = np.asarray(Wv, np.float32), np.asarray(bv, np.float32)
    Wo, bo = np.asarray(Wo, np.float32), np.asarray(bo, np.float32)

    if _NC is None:
        _NC = _build_program()
    in_maps = _prep_core_inputs(query, key, value, mask, Wq, bq, Wk, bk, Wv, Wo)
    res = run_bass_kernel_spmd(_NC, in_maps, core_ids=list(range(8)))
    if _results is not None:
        _results.append(res)

    # host epilogue: sum the two head-group partials; bv commutes with softmax
    # (rows sum to 1) so its contribution is Wo @ bv, plus the output bias bo.
    extra = (Wo.astype(np.float64) @ bv.astype(np.float64) + bo.astype(np.float64)).astype(
        np.float32
    )
    out = np.empty((B, S, E), dtype=np.float32)
    for b in range(B):
        out[b] = (
            res.results[2 * b]["out"] + res.results[2 * b + 1]["out"]
        ).T + extra
    return out
